# revision 15
# baseline (speedup 1.0000x reference)
"""4-bit groupwise-quantized linear layer (CLinear) on 8 Trainium2 NeuronCores.

Full-input contract: kernel(**inputs) takes the unsharded numpy inputs
  x      [4, 2048, 4096] fp32
  packed [4096, 64, 32]  int32 (byte values; hi nibble = first half of each
                                quant group, lo nibble = second half)
  mn     [4096, 64, 1]   fp32
  scale  [4096, 64, 1]   fp32
  bias   [4096]          fp32
and returns out[4, 2048, 4096] fp32 = x @ dequant(packed, mn, scale).T + bias.

Sharding: 2D grid over 8 cores — 4 token-row groups x 2 out-column groups.
Core (r, c) computes out[r*2048:(r+1)*2048, c*2048:(c+1)*2048] (transposed on
device, transposed back during host assembly). No collectives.

Device kernel per core (v2 design):
  - dequantize the 2048x4096 weight shard on-chip, n-tile granular (nibble
    extraction on DVE/GPSIMD, scale/offset via broadcast APs), bf16, and
    DMA-transpose each n-tile into a resident [k, n] SBUF tile;
  - stream x in 512-token blocks: fp32->bf16 (scalar engine), DMA-transpose
    to [k, m] layout;
  - matmuls with the weight n-tile stationary and tokens moving, fp32 PSUM
    accumulation -> psum holds out.T[n-tile, tokens]; bias is a free
    per-partition add during the scalar-engine PSUM eviction.
  - n-tile-granular dependencies let the dequant pipeline overlap the first
    matmul pass; x-prep for block q+1 overlaps pass q.
"""

import sys
from contextlib import ExitStack

import numpy as np

if "/opt/trn_rl_repo" not in sys.path:
    sys.path.insert(0, "/opt/trn_rl_repo")

import concourse.mybir as mybir
import concourse.tile as tile
from concourse import bacc
from concourse.bass_utils import run_bass_kernel_spmd

FP32 = mybir.dt.float32
BF16 = mybir.dt.bfloat16
I32 = mybir.dt.int32
U8 = mybir.dt.uint8
P = 128
GS = 64  # quant group size

# problem shape (hardcoded)
B, S, IN, OUT = 4, 2048, 4096, 4096
R_SHARDS, C_SHARDS = 4, 2
M_CORE = B * S // R_SHARDS      # 2048 tokens per core
N_CORE = OUT // C_SHARDS        # 2048 out features per core
MB = 256                        # tokens per matmul block


def _emit_kernel(tc, outs, ins, M, K, N, MB=512, G_CH=16):
    nc = tc.nc
    ctx = ExitStack()
    G = K // GS
    KT = K // P
    NT = N // P
    QT = M // MB
    MT_Q = MB // P
    G_CH = min(G_CH, G)
    GC = G // G_CH
    assert K % P == 0 and N % P == 0 and M % MB == 0 and MB % P == 0

    x_d = ins["x"]            # [M, K] fp32
    pk_d = ins["packed"]      # [N, G, 32] int32
    mn_d = ins["mn"]          # [N, G] fp32
    sc_d = ins["scale"]       # [N, G] fp32
    b_d = ins["bias"]         # [1, N] fp32
    out_d = outs["out"]       # [N, M] fp32  (transposed)

    with ctx:
        const = ctx.enter_context(tc.tile_pool(name="const", bufs=1))
        wres = ctx.enter_context(tc.tile_pool(name="wres", bufs=NT))
        deq = ctx.enter_context(tc.tile_pool(name="deq", bufs=2))
        xin = ctx.enter_context(tc.tile_pool(name="xin", bufs=2))
        xbp = ctx.enter_context(tc.tile_pool(name="xbp", bufs=1))
        xtp = ctx.enter_context(tc.tile_pool(name="xtp", bufs=2))
        outp = ctx.enter_context(tc.tile_pool(name="outp", bufs=2))
        psum = ctx.enter_context(tc.tile_pool(name="psum", bufs=3, space="PSUM"))

        # bias laid out [P, NT]: column nt holds bias[nt*128:(nt+1)*128]
        bias_pt = const.tile([P, NT], FP32)
        nc.sync.dma_start(out=bias_pt[:],
                          in_=b_d[:].rearrange("1 (t p) -> p t", p=P))

        def dequant_ntile(nt, eng):
            mn_t = deq.tile([P, G], FP32, tag="mn")
            nc.sync.dma_start(out=mn_t[:], in_=mn_d[nt * P:(nt + 1) * P])
            sc_t = deq.tile([P, G], FP32, tag="sc")
            nc.sync.dma_start(out=sc_t[:], in_=sc_d[nt * P:(nt + 1) * P])
            inv_t = deq.tile([P, G], FP32, tag="inv")
            nc.vector.reciprocal(inv_t[:], sc_t[:])

            # half-n-tile staging buffers -> two transposes per n-tile
            wt = wres.tile([P, KT, P], BF16, tag="wt")
            H = 2 if GC % 2 == 0 and GC >= 2 else 1
            G_H = G // H
            for h in range(H):
                wbf = deq.tile([P, G_H, GS], BF16, tag="wbf")
                for gc in range(GC // H):
                    g0 = h * G_H + gc * G_CH
                    gs_ = slice(g0, g0 + G_CH)
                    ls_ = slice(gc * G_CH, (gc + 1) * G_CH)
                    pk_t = deq.tile([P, G_CH, 32], I32, tag="pk")
                    nc.sync.dma_start(out=pk_t[:],
                                      in_=pk_d[nt * P:(nt + 1) * P, gs_])
                    pk8 = deq.tile([P, G_CH, 32], U8, tag="pk8")
                    eng.tensor_copy(pk8[:], pk_t[:])
                    vals = deq.tile([P, G_CH, GS], U8, tag="vals")
                    eng.tensor_scalar(
                        vals[:, :, 0:32], pk8[:], 4, None,
                        mybir.AluOpType.logical_shift_right)
                    eng.tensor_scalar(
                        vals[:, :, 32:64], pk8[:], 15, None,
                        mybir.AluOpType.bitwise_and)
                    inv_b = inv_t[:, gs_].unsqueeze(2).broadcast_to(
                        [P, G_CH, GS])
                    eng.tensor_tensor(wbf[:, ls_], vals[:], inv_b,
                                      mybir.AluOpType.mult)
                    mn_b = mn_t[:, gs_].unsqueeze(2).broadcast_to(
                        [P, G_CH, GS])
                    eng.tensor_tensor(wbf[:, ls_], wbf[:, ls_], mn_b,
                                      mybir.AluOpType.add)
                kt0 = h * (KT // H)
                nc.sync.dma_start_transpose(
                    wt[:, kt0:kt0 + KT // H, :],
                    wbf[:].rearrange("p g j -> p (g j)"))
            return wt

        def xprep(q, xT):
            for mt in range(MT_Q):
                m0 = q * MB + mt * P
                xb = xbp.tile([P, K], BF16, tag="xb")
                for xc in range(4):
                    xf = xin.tile([P, K // 4], FP32, tag="xf")
                    sl = slice(xc * K // 4, (xc + 1) * K // 4)
                    nc.sync.dma_start(out=xf[:], in_=x_d[m0:m0 + P, sl])
                    nc.scalar.activation(xb[:, sl], xf[:],
                                         mybir.ActivationFunctionType.Copy)
                nc.sync.dma_start_transpose(
                    xT[:, :, mt * P:(mt + 1) * P], xb[:])

        # emission order: first two weight n-tiles, then x-prep for block 0
        # (so its DMAs get early queue slots), then the remaining n-tiles.
        wts = []
        for nt in range(min(2, NT)):
            wts.append(dequant_ntile(nt, nc.vector))
        xT_cur = xtp.tile([P, KT, MB], BF16, tag="xT")
        xprep(0, xT_cur)
        for nt in range(min(2, NT), NT):
            wts.append(dequant_ntile(nt, nc.vector))

        for q in range(QT):
            xT_next = None
            if q + 1 < QT:
                xT_next = xtp.tile([P, KT, MB], BF16, tag="xT")
                xprep(q + 1, xT_next)
            for nt in range(NT):
                pt = psum.tile([P, MB], FP32, tag="pt")
                for k in range(KT):
                    nc.tensor.matmul(pt[:], lhsT=wts[nt][:, k, :],
                                     rhs=xT_cur[:, k, :],
                                     start=(k == 0), stop=(k == KT - 1))
                ot = outp.tile([P, MB], FP32, tag="ot")
                nc.scalar.activation(ot[:], pt[:],
                                     mybir.ActivationFunctionType.Identity,
                                     bias=bias_pt[:, nt:nt + 1])
                nc.sync.dma_start(
                    out=out_d[nt * P:(nt + 1) * P, q * MB:(q + 1) * MB],
                    in_=ot[:])
            xT_cur = xT_next


_CACHED = {}


def _build():
    if "nc" in _CACHED:
        return _CACHED["nc"]
    nc = bacc.Bacc("TRN2", target_bir_lowering=False, debug=False)
    tensors = {
        "x": nc.dram_tensor("x", [M_CORE, IN], FP32, kind="ExternalInput"),
        "packed": nc.dram_tensor("packed", [N_CORE, IN // GS, GS // 2], I32,
                                 kind="ExternalInput"),
        "mn": nc.dram_tensor("mn", [N_CORE, IN // GS], FP32,
                             kind="ExternalInput"),
        "scale": nc.dram_tensor("scale", [N_CORE, IN // GS], FP32,
                                kind="ExternalInput"),
        "bias": nc.dram_tensor("bias", [1, N_CORE], FP32,
                               kind="ExternalInput"),
        "out": nc.dram_tensor("out", [N_CORE, M_CORE], FP32,
                              kind="ExternalOutput"),
    }
    ins = {k: tensors[k].ap() for k in ("x", "packed", "mn", "scale", "bias")}
    outs = {"out": tensors["out"].ap()}
    with tile.TileContext(nc) as tc:
        _emit_kernel(tc, outs, ins, M=M_CORE, K=IN, N=N_CORE, MB=MB)
    nc.compile()
    _CACHED["nc"] = nc
    return nc


def kernel(x, packed, mn, scale, bias, _trace=False, _trace_kwargs=None):
    nc = _build()

    xf = np.ascontiguousarray(x.reshape(B * S, IN).astype(np.float32))
    in_maps = []
    for r in range(R_SHARDS):
        for c in range(C_SHARDS):
            in_maps.append({
                "x": xf[r * M_CORE:(r + 1) * M_CORE],
                "packed": np.ascontiguousarray(
                    packed[c * N_CORE:(c + 1) * N_CORE]),
                "mn": np.ascontiguousarray(
                    mn[c * N_CORE:(c + 1) * N_CORE, :, 0]),
                "scale": np.ascontiguousarray(
                    scale[c * N_CORE:(c + 1) * N_CORE, :, 0]),
                "bias": np.ascontiguousarray(
                    bias[c * N_CORE:(c + 1) * N_CORE].reshape(1, N_CORE)),
            })

    res = run_bass_kernel_spmd(
        nc, in_maps, core_ids=list(range(R_SHARDS * C_SHARDS)),
        trace=_trace, **(_trace_kwargs or {}))

    out = np.empty((B * S, OUT), np.float32)
    for r in range(R_SHARDS):
        for c in range(C_SHARDS):
            shard = res.results[r * C_SHARDS + c]["out"]  # [N_CORE, M_CORE]
            out[r * M_CORE:(r + 1) * M_CORE,
                c * N_CORE:(c + 1) * N_CORE] = shard.T
    kernel.last_exec_time_ns = res.exec_time_ns
    kernel.last_profile = res.profile_json
    return out.reshape(B, S, OUT)


# revision 17
# speedup vs baseline: 1.0082x; 1.0082x over previous
"""4-bit groupwise-quantized linear layer (CLinear) on 8 Trainium2 NeuronCores.

Full-input contract: kernel(**inputs) takes the unsharded numpy inputs
  x      [4, 2048, 4096] fp32
  packed [4096, 64, 32]  int32 (byte values; hi nibble = first half of each
                                quant group, lo nibble = second half)
  mn     [4096, 64, 1]   fp32
  scale  [4096, 64, 1]   fp32
  bias   [4096]          fp32
and returns out[4, 2048, 4096] fp32 = x @ dequant(packed, mn, scale).T + bias.

Sharding: 2D grid over 8 cores — 2 token-row groups x 4 out-column groups.
Core (r, c) computes out[r*4096:(r+1)*4096, c*1024:(c+1)*1024] (transposed on
device, transposed back during host assembly). No collectives.

Device kernel per core (v2 design):
  - dequantize the 2048x4096 weight shard on-chip, n-tile granular (nibble
    extraction on DVE/GPSIMD, scale/offset via broadcast APs), bf16, and
    DMA-transpose each n-tile into a resident [k, n] SBUF tile;
  - stream x in 512-token blocks: fp32->bf16 (scalar engine), DMA-transpose
    to [k, m] layout;
  - matmuls with the weight n-tile stationary and tokens moving, fp32 PSUM
    accumulation -> psum holds out.T[n-tile, tokens]; bias is a free
    per-partition add during the scalar-engine PSUM eviction.
  - n-tile-granular dependencies let the dequant pipeline overlap the first
    matmul pass; x-prep for block q+1 overlaps pass q.
"""

import sys
from contextlib import ExitStack

import numpy as np

if "/opt/trn_rl_repo" not in sys.path:
    sys.path.insert(0, "/opt/trn_rl_repo")

import concourse.mybir as mybir
import concourse.tile as tile
from concourse import bacc
from concourse.bass_utils import run_bass_kernel_spmd

FP32 = mybir.dt.float32
BF16 = mybir.dt.bfloat16
I32 = mybir.dt.int32
U8 = mybir.dt.uint8
P = 128
GS = 64  # quant group size

# problem shape (hardcoded)
B, S, IN, OUT = 4, 2048, 4096, 4096
R_SHARDS, C_SHARDS = 2, 4
M_CORE = B * S // R_SHARDS      # 2048 tokens per core
N_CORE = OUT // C_SHARDS        # 2048 out features per core
MB = 512                        # tokens per matmul block


def _emit_kernel(tc, outs, ins, M, K, N, MB=512, G_CH=16):
    nc = tc.nc
    ctx = ExitStack()
    G = K // GS
    KT = K // P
    NT = N // P
    QT = M // MB
    MT_Q = MB // P
    G_CH = min(G_CH, G)
    GC = G // G_CH
    assert K % P == 0 and N % P == 0 and M % MB == 0 and MB % P == 0

    x_d = ins["x"]            # [M, K] fp32
    pk_d = ins["packed"]      # [N, G, 32] int32
    mn_d = ins["mn"]          # [N, G] fp32
    sc_d = ins["scale"]       # [N, G] fp32
    b_d = ins["bias"]         # [1, N] fp32
    out_d = outs["out"]       # [N, M] fp32  (transposed)

    with ctx:
        const = ctx.enter_context(tc.tile_pool(name="const", bufs=1))
        wres = ctx.enter_context(tc.tile_pool(name="wres", bufs=NT))
        deq = ctx.enter_context(tc.tile_pool(name="deq", bufs=2))
        xin = ctx.enter_context(tc.tile_pool(name="xin", bufs=2))
        xbp = ctx.enter_context(tc.tile_pool(name="xbp", bufs=1))
        xtp = ctx.enter_context(tc.tile_pool(name="xtp", bufs=2))
        outp = ctx.enter_context(tc.tile_pool(name="outp", bufs=2))
        psum = ctx.enter_context(tc.tile_pool(name="psum", bufs=3, space="PSUM"))

        # bias laid out [P, NT]: column nt holds bias[nt*128:(nt+1)*128]
        bias_pt = const.tile([P, NT], FP32)
        nc.sync.dma_start(out=bias_pt[:],
                          in_=b_d[:].rearrange("1 (t p) -> p t", p=P))

        def dequant_ntile(nt, eng):
            mn_t = deq.tile([P, G], FP32, tag="mn")
            nc.sync.dma_start(out=mn_t[:], in_=mn_d[nt * P:(nt + 1) * P])
            sc_t = deq.tile([P, G], FP32, tag="sc")
            nc.sync.dma_start(out=sc_t[:], in_=sc_d[nt * P:(nt + 1) * P])
            inv_t = deq.tile([P, G], FP32, tag="inv")
            nc.vector.reciprocal(inv_t[:], sc_t[:])

            wbf = deq.tile([P, G, GS], BF16, tag="wbf")
            for gc in range(GC):
                gs_ = slice(gc * G_CH, (gc + 1) * G_CH)
                pk_t = deq.tile([P, G_CH, 32], I32, tag="pk")
                nc.sync.dma_start(out=pk_t[:],
                                  in_=pk_d[nt * P:(nt + 1) * P, gs_])
                pk8 = deq.tile([P, G_CH, 32], U8, tag="pk8")
                eng.tensor_copy(pk8[:], pk_t[:])
                vals = deq.tile([P, G_CH, GS], U8, tag="vals")
                eng.tensor_scalar(
                    vals[:, :, 0:32], pk8[:], 4, None,
                    mybir.AluOpType.logical_shift_right)
                eng.tensor_scalar(
                    vals[:, :, 32:64], pk8[:], 15, None,
                    mybir.AluOpType.bitwise_and)
                inv_b = inv_t[:, gs_].unsqueeze(2).broadcast_to([P, G_CH, GS])
                eng.tensor_tensor(wbf[:, gs_], vals[:], inv_b,
                                  mybir.AluOpType.mult)
                mn_b = mn_t[:, gs_].unsqueeze(2).broadcast_to([P, G_CH, GS])
                eng.tensor_tensor(wbf[:, gs_], wbf[:, gs_], mn_b,
                                  mybir.AluOpType.add)

            wt = wres.tile([P, KT, P], BF16, tag="wt")
            nc.sync.dma_start_transpose(
                wt[:], wbf[:].rearrange("p g j -> p (g j)"))
            return wt

        wts = []
        for nt in range(NT):
            wts.append(dequant_ntile(nt, nc.vector))

        def xprep(q, xT):
            for mt in range(MT_Q):
                m0 = q * MB + mt * P
                xb = xbp.tile([P, K], BF16, tag="xb")
                for xc in range(4):
                    xf = xin.tile([P, K // 4], FP32, tag="xf")
                    sl = slice(xc * K // 4, (xc + 1) * K // 4)
                    nc.sync.dma_start(out=xf[:], in_=x_d[m0:m0 + P, sl])
                    nc.scalar.activation(xb[:, sl], xf[:],
                                         mybir.ActivationFunctionType.Copy)
                nc.sync.dma_start_transpose(
                    xT[:, :, mt * P:(mt + 1) * P], xb[:])

        xT_cur = xtp.tile([P, KT, MB], BF16, tag="xT")
        xprep(0, xT_cur)
        for q in range(QT):
            xT_next = None
            if q + 1 < QT:
                xT_next = xtp.tile([P, KT, MB], BF16, tag="xT")
                xprep(q + 1, xT_next)
            for nt in range(NT):
                pt = psum.tile([P, MB], FP32, tag="pt")
                for k in range(KT):
                    nc.tensor.matmul(pt[:], lhsT=wts[nt][:, k, :],
                                     rhs=xT_cur[:, k, :],
                                     start=(k == 0), stop=(k == KT - 1))
                ot = outp.tile([P, MB], FP32, tag="ot")
                nc.scalar.activation(ot[:], pt[:],
                                     mybir.ActivationFunctionType.Identity,
                                     bias=bias_pt[:, nt:nt + 1])
                nc.sync.dma_start(
                    out=out_d[nt * P:(nt + 1) * P, q * MB:(q + 1) * MB],
                    in_=ot[:])
            xT_cur = xT_next


_CACHED = {}


def _build():
    if "nc" in _CACHED:
        return _CACHED["nc"]
    nc = bacc.Bacc("TRN2", target_bir_lowering=False, debug=False)
    tensors = {
        "x": nc.dram_tensor("x", [M_CORE, IN], FP32, kind="ExternalInput"),
        "packed": nc.dram_tensor("packed", [N_CORE, IN // GS, GS // 2], I32,
                                 kind="ExternalInput"),
        "mn": nc.dram_tensor("mn", [N_CORE, IN // GS], FP32,
                             kind="ExternalInput"),
        "scale": nc.dram_tensor("scale", [N_CORE, IN // GS], FP32,
                                kind="ExternalInput"),
        "bias": nc.dram_tensor("bias", [1, N_CORE], FP32,
                               kind="ExternalInput"),
        "out": nc.dram_tensor("out", [N_CORE, M_CORE], FP32,
                              kind="ExternalOutput"),
    }
    ins = {k: tensors[k].ap() for k in ("x", "packed", "mn", "scale", "bias")}
    outs = {"out": tensors["out"].ap()}
    with tile.TileContext(nc) as tc:
        _emit_kernel(tc, outs, ins, M=M_CORE, K=IN, N=N_CORE, MB=MB)
    nc.compile()
    _CACHED["nc"] = nc
    return nc


def kernel(x, packed, mn, scale, bias, _trace=False, _trace_kwargs=None):
    nc = _build()

    xf = np.ascontiguousarray(x.reshape(B * S, IN).astype(np.float32))
    in_maps = []
    for r in range(R_SHARDS):
        for c in range(C_SHARDS):
            in_maps.append({
                "x": xf[r * M_CORE:(r + 1) * M_CORE],
                "packed": np.ascontiguousarray(
                    packed[c * N_CORE:(c + 1) * N_CORE]),
                "mn": np.ascontiguousarray(
                    mn[c * N_CORE:(c + 1) * N_CORE, :, 0]),
                "scale": np.ascontiguousarray(
                    scale[c * N_CORE:(c + 1) * N_CORE, :, 0]),
                "bias": np.ascontiguousarray(
                    bias[c * N_CORE:(c + 1) * N_CORE].reshape(1, N_CORE)),
            })

    res = run_bass_kernel_spmd(
        nc, in_maps, core_ids=list(range(R_SHARDS * C_SHARDS)),
        trace=_trace, **(_trace_kwargs or {}))

    out = np.empty((B * S, OUT), np.float32)
    for r in range(R_SHARDS):
        for c in range(C_SHARDS):
            shard = res.results[r * C_SHARDS + c]["out"]  # [N_CORE, M_CORE]
            out[r * M_CORE:(r + 1) * M_CORE,
                c * N_CORE:(c + 1) * N_CORE] = shard.T
    kernel.last_exec_time_ns = res.exec_time_ns
    kernel.last_profile = res.profile_json
    return out.reshape(B, S, OUT)
